# revision 14
# baseline (speedup 1.0000x reference)
"""Trainium2 Bass kernel for nn_CorrClassLoss.

Reference computation (B=4, C=19, H=512, W=1024, N=5000, IGNORE=255):
  ref_class = argmax_c inputs_ref[b].reshape(C, H*W)      # flat W-major
  lin_ref   = 512*y_ref + x_ref    (NOTE: linearized with H, kept faithfully)
  lin_other = 512*y_other + x_other
  gathered  = ref_class[b, lin_ref]
  target[b, lin_other] = gathered  (scatter, last write wins; rest IGNORE)
  loss = mean over non-ignored pixels of -log_softmax(inputs_other)[b, target, px]

Only flat positions [0, 262144) are touched; at most N unique scatter dests
per batch contribute:

  loss = -(1/cnt) * sum over unique dests d (last writer j, src s_j) of
         [ x_other[b, cls(s_j), d] - ln(sum_c exp(x_other[b, c, d])) ]
  cls(s) = argmax_c x_ref[b, c, s],  cnt = total unique dests.

Strategy (8 cores, data-parallel over (batch, half-of-sources)). Host does
index-only math (dedup last-wins, core split, window/parity sort, idx
packing) plus pure relayout of image data (pixel-major transpose into
64-slot rows; two pixels share one 512B table row so one int16-indexed
32K-row gather window covers 64K pixels). Device per core:
  - 4 (dest) + 4 (source) InstDMAGatherAnt fetch two-pixel rows into
    window/parity-sorted slot spaces (slot j = [j%128, j//128]); per-parity
    strided views compact the valid 19 channels into packed tiles.
  - dest side: ln(sum_c exp(.)) (masked) -> term2 partials; the packed
    dest rows are dumped to a DRAM scratch (one strided DMA) and regathered
    (3x InstDMAGatherAnt, chunked under the 1024-descriptor carveout) into
    source-slot order, where the argmax one-hot (max + is_ge) pairs them:
    term1 partials = sum one-hot . other_vec (mask folded into the one-hot).
  Output [128, Gs+Gd] = (term1 partials | term2 partials); host sums and
  computes loss = -(sum t1 - sum t2) / cnt.
"""

import sys

if "/opt/trn_rl_repo" not in sys.path:
    sys.path.insert(0, "/opt/trn_rl_repo")

import numpy as np

B, C, H, W = 4, 19, 512, 1024
HW = H * W                 # 524288
NPIX = 262144              # touched flat range [0, 262144)
NPIX_H = NPIX // 2         # 131072 source pixels per core
N = 5000
NCORES = 8

P = 128                    # partitions
E128 = 128                 # f32 slots per two-pixel table row (512B)
W_ROWS = 32768             # rows per dma_gather window (int16 idx range)
NW_S = NPIX_H // 2 // W_ROWS   # 2 source windows per core
NW_D = NPIX // 2 // W_ROWS     # 4 dest windows per core
NG_S = NW_S * 2            # source (window, parity) groups
NG_D = NW_D * 2            # dest (window, parity) groups
RCH = 8                    # gather chunk columns (1024 idx <= carveout)

_programs = {}


def _build_program(key):
    import concourse.bass as bass
    import concourse.bacc as bacc
    import concourse.mybir as mybir
    import concourse.tile as tile

    GS = list(key[0])          # columns per source (window, parity) group
    GD = list(key[1])          # columns per dest (window, parity) group
    Gs, Gd = sum(GS), sum(GD)
    offs = np.concatenate([[0], np.cumsum(GS)]).astype(int)
    offd = np.concatenate([[0], np.cumsum(GD)]).astype(int)

    nc = bacc.Bacc("TRN2", target_bir_lowering=False, debug=False,
                   num_devices=NCORES)

    refq = nc.dram_tensor("refq", [NPIX_H // 4, E128], mybir.dt.bfloat16,
                          kind="ExternalInput")
    oth2 = nc.dram_tensor("oth2", [NPIX // 2, E128], mybir.dt.float32,
                          kind="ExternalInput")
    # idx streams (int16, 16-wrapped, replicated x8): d gates the first
    # gathers so it uploads alone; s and r follow
    idx_d = nc.dram_tensor("idx_d", [P, Gd * 8], mybir.dt.int16,
                           kind="ExternalInput")
    idx_sr = nc.dram_tensor("idx_sr", [P, 2 * Gs * 8], mybir.dt.int16,
                            kind="ExternalInput")
    # valid masks: [pm_d | pm_s]
    pmio = nc.dram_tensor("pmio", [P, Gd + Gs], mybir.dt.float32,
                          kind="ExternalInput")
    scratch = nc.dram_tensor("scratch", [P * Gd, 64], mybir.dt.float32,
                             kind="Internal")
    out = nc.dram_tensor("out", [P, Gs + Gd], mybir.dt.float32,
                         kind="ExternalOutput")

    with tile.TileContext(nc) as tc:
        with (
            tc.tile_pool(name="gb", bufs=1) as gb,
            tc.tile_pool(name="cons", bufs=1) as cons,
        ):
            ixd = gb.tile([P, Gd * 8], mybir.dt.int16)
            nc.sync.dma_start(out=ixd[:], in_=idx_d[:, :])
            ixsr = gb.tile([P, 2 * Gs * 8], mybir.dt.int16)
            nc.sync.dma_start(out=ixsr[:], in_=idx_sr[:, :])
            pm = gb.tile([P, Gd + Gs], mybir.dt.float32)
            nc.sync.dma_start(out=pm[:], in_=pmio[:, :])

            OTH = gb.tile([P, Gd * E128], mybir.dt.float32)
            REF = gb.tile([P, Gs * E128], mybir.dt.bfloat16)
            R2S = gb.tile([P, Gs * 64], mybir.dt.float32)

            def win_gathers(table, ixt, base8, offg, tile_out):
                """One dma_gather per (window, <=RCH column chunk); groups
                2w and 2w+1 belong to window w."""
                for wdx in range(len(offg) // 2):
                    lo, hi = int(offg[2 * wdx]), int(offg[2 * wdx + 2])
                    c0 = lo
                    while c0 < hi:
                        c1 = min(c0 + RCH, hi)
                        nc.gpsimd.dma_gather(
                            out_ap=tile_out[
                                :, c0 * E128:c1 * E128].rearrange(
                                "p (g c) -> p g c", c=E128),
                            in_ap=table[wdx * W_ROWS:(wdx + 1) * W_ROWS, :],
                            idxs_ap=ixt[:, base8 + c0 * 8:base8 + c1 * 8],
                            num_idxs=(c1 - c0) * P,
                            num_idxs_reg=(c1 - c0) * P,
                            elem_size=E128,
                        )
                        c0 = c1

            def compact(src_tile, dst_tile, offg, g0=0, nsub=2):
                """Per-(window, parity) strided copy of the valid 19
                channels into the packed [P, G*19] tile; a row holds nsub
                pixels and the group parity (absolute index g0+g mod nsub)
                selects which (128//nsub)-slot sub-row. Converts dtype."""
                for g in range(len(offg) - 1):
                    lo, hi = int(offg[g]), int(offg[g + 1])
                    if hi == lo:
                        continue
                    base = ((g0 + g) % nsub) * (E128 // nsub)
                    sv = src_tile[:, lo * E128:hi * E128].rearrange(
                        "p (g c) -> p g c", c=E128)[:, :, base:base + 19]
                    dv = dst_tile[:, lo * 19:hi * 19].rearrange(
                        "p (g c) -> p g c", c=19)
                    nc.vector.tensor_copy(out=dv, in_=sv)

            # d-side first: it feeds the compaction -> dump -> regather chain
            win_gathers(oth2, ixd, 0, offd, OTH)
            # s-side: one bf16 4-pixel-row window covers all source pixels
            c0 = 0
            while c0 < Gs:
                c1 = min(c0 + RCH, Gs)
                nc.gpsimd.dma_gather(
                    out_ap=REF[:, c0 * E128:c1 * E128].rearrange(
                        "p (g c) -> p g c", c=E128),
                    in_ap=refq[:, :],
                    idxs_ap=ixsr[:, c0 * 8:c1 * 8],
                    num_idxs=(c1 - c0) * P,
                    num_idxs_reg=(c1 - c0) * P,
                    elem_size=E128,
                )
                c0 = c1

            # compact dest rows to packed [P, Gd*19]; dump each dest
            # window's columns to scratch rows (p*Gd + g) as soon as its
            # gather lands, so the last (smallest) dump gates the regather
            # as briefly as possible
            OPK = gb.tile([P, Gd * 19], mybir.dt.float32)
            OPKv = OPK[:].rearrange("p (g c) -> p g c", c=19)
            scr3 = scratch.rearrange("(p g) c -> p g c", g=Gd)
            for m in range(NW_D):
                lo, hi = int(offd[2 * m]), int(offd[2 * m + 2])
                if hi == lo:
                    continue
                compact(OTH, OPK, offd[2 * m:2 * m + 3], g0=2 * m, nsub=2)
                nc.sync.dma_start(out=scr3[:, lo:hi, 0:19],
                                  in_=OPKv[:, lo:hi, :])

            # term2 in d-space: ln(sum_c exp(other_vec)), masked
            e2 = gb.tile([P, Gd * 19], mybir.dt.float32)
            e2v = e2[:].rearrange("p (g c) -> p g c", c=19)
            nc.scalar.activation(e2v, OPKv, mybir.ActivationFunctionType.Exp)
            S2 = gb.tile([P, Gd], mybir.dt.float32)
            nc.vector.tensor_reduce(out=S2[:], in_=e2v,
                                    axis=mybir.AxisListType.X,
                                    op=mybir.AluOpType.add)
            TG = cons.tile([P, Gs + Gd], mybir.dt.float32)
            L2 = TG[:, Gs:]
            nc.scalar.activation(L2, S2[:], mybir.ActivationFunctionType.Ln)
            nc.vector.tensor_tensor(out=L2, in0=L2, in1=pm[:, 0:Gd],
                                    op=mybir.AluOpType.mult)

            # s-space argmax one-hot (pm_s folded in)
            RPK = gb.tile([P, Gs * 19], mybir.dt.float32)
            compact(REF, RPK, offs, g0=0, nsub=4)
            RPKv = RPK[:].rearrange("p (g c) -> p g c", c=19)
            m2 = gb.tile([P, Gs], mybir.dt.float32)
            nc.vector.tensor_reduce(out=m2[:], in_=RPKv,
                                    axis=mybir.AxisListType.X,
                                    op=mybir.AluOpType.max)
            eq2 = gb.tile([P, Gs * 19], mybir.dt.float32)
            eq2v = eq2[:].rearrange("p (g c) -> p g c", c=19)
            nc.vector.tensor_tensor(
                out=eq2v, in0=RPKv,
                in1=m2[:, :, None].to_broadcast([P, Gs, 19]),
                op=mybir.AluOpType.is_ge,
            )
            nc.vector.tensor_tensor(
                out=eq2v, in0=eq2v,
                in1=pm[:, Gd:, None].to_broadcast([P, Gs, 19]),
                op=mybir.AluOpType.mult,
            )

            # route other rows into s-slot order (chunked regather), pairing
            # each chunk as soon as it lands
            rbase = Gs * 8
            t1g = TG[:, 0:Gs]
            for lo in range(0, Gs, RCH):
                hi = min(lo + RCH, Gs)
                w = hi - lo
                nc.gpsimd.dma_gather(
                    out_ap=R2S[:, lo * 64:hi * 64].rearrange(
                        "p (g c) -> p g c", c=64),
                    in_ap=scratch[:, :],
                    idxs_ap=ixsr[:, rbase + lo * 8:rbase + hi * 8],
                    num_idxs=w * P,
                    num_idxs_reg=w * P,
                    elem_size=64,
                )
                R2v = R2S[:, lo * 64:hi * 64].rearrange(
                    "p (g c) -> p g c", c=64)[:, :, 0:19]
                eqc = eq2[:, lo * 19:hi * 19].rearrange(
                    "p (g c) -> p g c", c=19)
                nc.vector.tensor_tensor(out=eqc, in0=eqc, in1=R2v,
                                        op=mybir.AluOpType.mult)
                nc.vector.tensor_reduce(out=t1g[:, lo:hi], in_=eqc,
                                        axis=mybir.AxisListType.X,
                                        op=mybir.AluOpType.add)

            nc.sync.dma_start(out=out[:, :], in_=TG[:])

    nc.finalize()
    return nc


def _get_program(key):
    if key not in _programs:
        _programs[key] = _build_program(key)
    return _programs[key]


def _host_prep(inds_ref, inds_other):
    """Index-only host math: dedup scatter (last wins), split per core,
    sort both slot spaces by (two-pixel-row window, parity)."""
    ir = np.asarray(inds_ref).astype(np.int64)      # [B, 2, N]
    io = np.asarray(inds_other).astype(np.int64)
    valid = ((ir[:, 0] >= 0) & (ir[:, 0] < W) & (ir[:, 1] >= 0) & (ir[:, 1] < H)
             & (io[:, 0] >= 0) & (io[:, 0] < W) & (io[:, 1] >= 0)
             & (io[:, 1] < H))                       # [B, N]
    lin_ref = H * ir[:, 1] + ir[:, 0]                # [B, N]
    lin_other = H * io[:, 1] + io[:, 0]

    per_core = []
    count = 0
    for b in range(B):
        v = valid[b]
        lo = lin_other[b][v]
        lr = np.clip(lin_ref[b][v], 0, HW - 1)
        u, first_rev = np.unique(lo[::-1], return_index=True)
        d_arr = u.astype(np.int64)
        s_arr = lr[len(lo) - 1 - first_rev].astype(np.int64)
        count += len(u)
        for h in range(2):
            sel = (s_arr // NPIX_H) == h
            s_local = s_arr[sel] - h * NPIX_H
            d_sel = d_arr[sel]
            # source: 4px bf16 rows, one window -> group = pixel & 3
            # dest: 2px f32 rows -> group = (row window, pixel parity)
            gs = s_local & 3
            gd = (d_sel >> 16) * 2 + (d_sel & 1)
            s_ord = np.argsort(gs, kind='stable')
            d_ord = np.argsort(gd, kind='stable')
            per_core.append({
                "s": s_local[s_ord], "d": d_sel[d_ord],
                # for each s-sorted position, the d-sorted position of the
                # same correspondence (routing for the regather)
                "route": np.argsort(d_ord, kind='stable')[s_ord],
                "ngs": np.bincount(gs[s_ord], minlength=NG_S).astype(int),
                "ngd": np.bincount(gd[d_ord], minlength=NG_D).astype(int),
            })
    return per_core, count


def _plan(per_core):
    ngs = np.stack([pc["ngs"] for pc in per_core])
    ngd = np.stack([pc["ngd"] for pc in per_core])
    GS = np.maximum(1, -(-ngs.max(axis=0) // P))
    GD = np.maximum(1, -(-ngd.max(axis=0) // P))
    return (tuple(int(g) for g in GS), tuple(int(g) for g in GD))


def _wrap16(vals, ncols8):
    outp = np.zeros((16, ncols8), dtype=np.int16)
    j = np.arange(len(vals))
    outp[j % 16, j // 16] = vals.astype(np.int16)
    return outp


def _pack_core(pc, key):
    GS, GD = np.asarray(key[0]), np.asarray(key[1])
    Gs, Gd = int(GS.sum()), int(GD.sum())
    offs = np.concatenate([[0], np.cumsum(GS)]).astype(int)
    offd = np.concatenate([[0], np.cumsum(GD)]).astype(int)

    d_stream = np.zeros(Gd * P, dtype=np.int64)
    pm_d = np.zeros((P, Gd), dtype=np.float32)
    dpos2lin = np.zeros(len(pc["d"]), dtype=np.int64)
    pos = 0
    for g in range(NG_D):
        n = int(pc["ngd"][g])
        jj = np.arange(n)
        # window-local two-pixel row index
        d_stream[offd[g] * P + jj] = (pc["d"][pos:pos + n] >> 1) - \
            (g // 2) * W_ROWS
        gcol = offd[g] + jj // P
        pm_d[jj % P, gcol] = 1.0
        dpos2lin[pos:pos + n] = (jj % P) * Gd + gcol
        pos += n

    s_stream = np.zeros(Gs * P, dtype=np.int64)
    r_stream = np.zeros(Gs * P, dtype=np.int64)
    pm_s = np.zeros((P, Gs), dtype=np.float32)
    pos = 0
    for g in range(NG_S):
        n = int(pc["ngs"][g])
        jj = np.arange(n)
        s_stream[offs[g] * P + jj] = pc["s"][pos:pos + n] >> 2
        r_stream[offs[g] * P + jj] = dpos2lin[pc["route"][pos:pos + n]]
        pm_s[jj % P, offs[g] + jj // P] = 1.0
        pos += n

    idx_d = np.tile(_wrap16(d_stream, Gd * 8), (8, 1))
    idx_sr = np.tile(np.concatenate([
        _wrap16(s_stream, Gs * 8),
        _wrap16(r_stream, Gs * 8),
    ], axis=1), (8, 1))
    pmio = np.concatenate([pm_d, pm_s], axis=1)
    return idx_d, idx_sr, pmio


def _make_in_maps(inputs_ref, inputs_other, per_core, key):
    ref_flat = inputs_ref.reshape(B, C, HW)
    other_flat = inputs_other.reshape(B, C, HW)
    other_cache = {}
    in_maps = []
    import ml_dtypes
    for ci, pc in enumerate(per_core):
        b, h = ci // 2, ci % 2
        refq = np.empty((NPIX_H, 32), dtype=ml_dtypes.bfloat16)
        refq[:, :C] = ref_flat[b, :, h * NPIX_H:(h + 1) * NPIX_H].T
        if b not in other_cache:
            o64 = np.empty((NPIX, 64), dtype=np.float32)
            o64[:, :C] = other_flat[b, :, :NPIX].T
            other_cache[b] = o64.reshape(NPIX // 2, E128)
        idx_d, idx_sr, pmio = _pack_core(pc, key)
        in_maps.append({
            "refq": refq.reshape(NPIX_H // 4, E128),
            "oth2": other_cache[b],
            "idx_d": idx_d,
            "idx_sr": idx_sr,
            "pmio": pmio,
        })
    return in_maps


def kernel(inputs_ref, inputs_other, inds_ref, inds_other, weights):
    from concourse.bass_utils import run_bass_kernel_spmd

    inputs_ref = np.asarray(inputs_ref, dtype=np.float32)
    inputs_other = np.asarray(inputs_other, dtype=np.float32)

    per_core, count = _host_prep(inds_ref, inds_other)
    key = _plan(per_core)
    nc = _get_program(key)

    in_maps = _make_in_maps(inputs_ref, inputs_other, per_core, key)
    res = run_bass_kernel_spmd(nc, in_maps, core_ids=list(range(NCORES)))
    total = 0.0
    Gs = sum(key[0])
    for r in res.results:
        o = np.asarray(r["out"], dtype=np.float64)
        total += o[:, :Gs].sum() - o[:, Gs:].sum()
    loss = -total / max(count, 1)
    return np.float32(loss)


# revision 15
# speedup vs baseline: 1.0637x; 1.0637x over previous
"""Trainium2 Bass kernel for nn_CorrClassLoss.

Reference computation (B=4, C=19, H=512, W=1024, N=5000, IGNORE=255):
  ref_class = argmax_c inputs_ref[b].reshape(C, H*W)      # flat W-major
  lin_ref   = 512*y_ref + x_ref    (NOTE: linearized with H, kept faithfully)
  lin_other = 512*y_other + x_other
  gathered  = ref_class[b, lin_ref]
  target[b, lin_other] = gathered  (scatter, last write wins; rest IGNORE)
  loss = mean over non-ignored pixels of -log_softmax(inputs_other)[b, target, px]

Only flat positions [0, 262144) are touched; at most N unique scatter dests
per batch contribute:

  loss = -(1/cnt) * sum over unique dests d (last writer j, src s_j) of
         [ x_other[b, cls(s_j), d] - ln(sum_c exp(x_other[b, c, d])) ]
  cls(s) = argmax_c x_ref[b, c, s],  cnt = total unique dests.

Strategy (8 cores, data-parallel over (batch, half-of-sources)). Host does
index-only math (dedup last-wins, core split, window/parity sort, idx
packing) plus pure relayout of image data (pixel-major transpose into
64-slot rows; two pixels share one 512B table row so one int16-indexed
32K-row gather window covers 64K pixels). Device per core:
  - 4 (dest) + 4 (source) InstDMAGatherAnt fetch two-pixel rows into
    window/parity-sorted slot spaces (slot j = [j%128, j//128]); per-parity
    strided views compact the valid 19 channels into packed tiles.
  - dest side: ln(sum_c exp(.)) (masked) -> term2 partials; the packed
    dest rows are dumped to a DRAM scratch (one strided DMA) and regathered
    (3x InstDMAGatherAnt, chunked under the 1024-descriptor carveout) into
    source-slot order, where the argmax one-hot (max + is_ge) pairs them:
    term1 partials = sum one-hot . other_vec (mask folded into the one-hot).
  Output [128, Gs+Gd] = (term1 partials | term2 partials); host sums and
  computes loss = -(sum t1 - sum t2) / cnt.
"""

import sys

if "/opt/trn_rl_repo" not in sys.path:
    sys.path.insert(0, "/opt/trn_rl_repo")

import numpy as np

B, C, H, W = 4, 19, 512, 1024
HW = H * W                 # 524288
NPIX = 262144              # touched flat range [0, 262144)
NPIX_H = NPIX // 2         # 131072 source pixels per core
N = 5000
NCORES = 8

P = 128                    # partitions
E128 = 128                 # f32 slots per two-pixel table row (512B)
W_ROWS = 32768             # rows per dma_gather window (int16 idx range)
NW_S = NPIX_H // 2 // W_ROWS   # 2 source windows per core
NW_D = NPIX // 2 // W_ROWS     # 4 dest windows per core
NG_S = NW_S * 2            # source (window, parity) groups
NG_D = NW_D * 2            # dest (window, parity) groups
RCH = 8                    # gather chunk columns (1024 idx <= carveout)

_programs = {}


def _build_program(key):
    import concourse.bass as bass
    import concourse.bacc as bacc
    import concourse.mybir as mybir
    import concourse.tile as tile

    GS = list(key[0])          # columns per source (window, parity) group
    GD = list(key[1])          # columns per dest (window, parity) group
    Gs, Gd = sum(GS), sum(GD)
    offs = np.concatenate([[0], np.cumsum(GS)]).astype(int)
    offd = np.concatenate([[0], np.cumsum(GD)]).astype(int)

    nc = bacc.Bacc("TRN2", target_bir_lowering=False, debug=False,
                   num_devices=NCORES)

    ref2 = nc.dram_tensor("ref2", [NPIX_H // 2, E128], mybir.dt.float32,
                          kind="ExternalInput")
    oth2 = nc.dram_tensor("oth2", [NPIX // 2, E128], mybir.dt.float32,
                          kind="ExternalInput")
    # idx streams (int16, 16-wrapped, replicated x8): d gates the first
    # gathers so it uploads alone; s and r follow
    idx_d = nc.dram_tensor("idx_d", [P, Gd * 8], mybir.dt.int16,
                           kind="ExternalInput")
    idx_sr = nc.dram_tensor("idx_sr", [P, 2 * Gs * 8], mybir.dt.int16,
                            kind="ExternalInput")
    # valid masks: [pm_d | pm_s]
    pmio = nc.dram_tensor("pmio", [P, Gd + Gs], mybir.dt.float32,
                          kind="ExternalInput")
    scratch = nc.dram_tensor("scratch", [P * Gd, 64], mybir.dt.float32,
                             kind="Internal")
    out = nc.dram_tensor("out", [P, Gs + Gd], mybir.dt.float32,
                         kind="ExternalOutput")

    with tile.TileContext(nc) as tc:
        with (
            tc.tile_pool(name="gb", bufs=1) as gb,
            tc.tile_pool(name="cons", bufs=1) as cons,
        ):
            ixd = gb.tile([P, Gd * 8], mybir.dt.int16)
            nc.sync.dma_start(out=ixd[:], in_=idx_d[:, :])
            ixsr = gb.tile([P, 2 * Gs * 8], mybir.dt.int16)
            nc.sync.dma_start(out=ixsr[:], in_=idx_sr[:, :])
            pm = gb.tile([P, Gd + Gs], mybir.dt.float32)
            nc.sync.dma_start(out=pm[:], in_=pmio[:, :])

            OTH = gb.tile([P, Gd * E128], mybir.dt.float32)
            REF = gb.tile([P, Gs * E128], mybir.dt.float32)
            R2S = gb.tile([P, Gs * 64], mybir.dt.float32)

            def win_gathers(table, ixt, base8, offg, tile_out):
                """One dma_gather per (window, <=RCH column chunk); groups
                2w and 2w+1 belong to window w."""
                for wdx in range(len(offg) // 2):
                    lo, hi = int(offg[2 * wdx]), int(offg[2 * wdx + 2])
                    c0 = lo
                    while c0 < hi:
                        c1 = min(c0 + RCH, hi)
                        nc.gpsimd.dma_gather(
                            out_ap=tile_out[
                                :, c0 * E128:c1 * E128].rearrange(
                                "p (g c) -> p g c", c=E128),
                            in_ap=table[wdx * W_ROWS:(wdx + 1) * W_ROWS, :],
                            idxs_ap=ixt[:, base8 + c0 * 8:base8 + c1 * 8],
                            num_idxs=(c1 - c0) * P,
                            num_idxs_reg=(c1 - c0) * P,
                            elem_size=E128,
                        )
                        c0 = c1

            def compact(src_tile, dst_tile, offg, col0=None):
                """Per-(window, parity) strided copy of the valid 19
                channels into the packed [P, G*19] tile (parity selects
                the 64-slot half of the two-pixel row). offg holds absolute
                column offsets; group parity alternates from the parity of
                the first group's index, which is even for both slot spaces
                and for both dump halves (NG_D//2 is even)."""
                for g in range(len(offg) - 1):
                    lo, hi = int(offg[g]), int(offg[g + 1])
                    if hi == lo:
                        continue
                    base = (g & 1) * 64
                    sv = src_tile[:, lo * E128:hi * E128].rearrange(
                        "p (g c) -> p g c", c=E128)[:, :, base:base + 19]
                    dv = dst_tile[:, lo * 19:hi * 19].rearrange(
                        "p (g c) -> p g c", c=19)
                    nc.vector.tensor_copy(out=dv, in_=sv)

            # d-side first: it feeds the compaction -> dump -> regather chain
            win_gathers(oth2, ixd, 0, offd, OTH)
            win_gathers(ref2, ixsr, 0, offs, REF)

            # compact dest rows to packed [P, Gd*19]; dump each dest
            # window's columns to scratch rows (p*Gd + g) as soon as its
            # gather lands, so the last (smallest) dump gates the regather
            # as briefly as possible
            OPK = gb.tile([P, Gd * 19], mybir.dt.float32)
            OPKv = OPK[:].rearrange("p (g c) -> p g c", c=19)
            scr3 = scratch.rearrange("(p g) c -> p g c", g=Gd)
            for m in range(NW_D):
                lo, hi = int(offd[2 * m]), int(offd[2 * m + 2])
                if hi == lo:
                    continue
                compact(OTH, OPK, offd[2 * m:2 * m + 3])
                nc.sync.dma_start(out=scr3[:, lo:hi, 0:19],
                                  in_=OPKv[:, lo:hi, :])

            # term2 in d-space: ln(sum_c exp(other_vec)), masked
            e2 = gb.tile([P, Gd * 19], mybir.dt.float32)
            e2v = e2[:].rearrange("p (g c) -> p g c", c=19)
            nc.scalar.activation(e2v, OPKv, mybir.ActivationFunctionType.Exp)
            S2 = gb.tile([P, Gd], mybir.dt.float32)
            nc.vector.tensor_reduce(out=S2[:], in_=e2v,
                                    axis=mybir.AxisListType.X,
                                    op=mybir.AluOpType.add)
            TG = cons.tile([P, Gs + Gd], mybir.dt.float32)
            L2 = TG[:, Gs:]
            nc.scalar.activation(L2, S2[:], mybir.ActivationFunctionType.Ln)
            nc.vector.tensor_tensor(out=L2, in0=L2, in1=pm[:, 0:Gd],
                                    op=mybir.AluOpType.mult)

            # s-space argmax one-hot (pm_s folded in)
            RPK = gb.tile([P, Gs * 19], mybir.dt.float32)
            compact(REF, RPK, offs)
            RPKv = RPK[:].rearrange("p (g c) -> p g c", c=19)
            m2 = gb.tile([P, Gs], mybir.dt.float32)
            nc.vector.tensor_reduce(out=m2[:], in_=RPKv,
                                    axis=mybir.AxisListType.X,
                                    op=mybir.AluOpType.max)
            eq2 = gb.tile([P, Gs * 19], mybir.dt.float32)
            eq2v = eq2[:].rearrange("p (g c) -> p g c", c=19)
            nc.vector.tensor_tensor(
                out=eq2v, in0=RPKv,
                in1=m2[:, :, None].to_broadcast([P, Gs, 19]),
                op=mybir.AluOpType.is_ge,
            )
            nc.vector.tensor_tensor(
                out=eq2v, in0=eq2v,
                in1=pm[:, Gd:, None].to_broadcast([P, Gs, 19]),
                op=mybir.AluOpType.mult,
            )

            # route other rows into s-slot order (chunked regather), pairing
            # each chunk as soon as it lands
            rbase = Gs * 8
            t1g = TG[:, 0:Gs]
            for lo in range(0, Gs, RCH):
                hi = min(lo + RCH, Gs)
                w = hi - lo
                nc.gpsimd.dma_gather(
                    out_ap=R2S[:, lo * 64:hi * 64].rearrange(
                        "p (g c) -> p g c", c=64),
                    in_ap=scratch[:, :],
                    idxs_ap=ixsr[:, rbase + lo * 8:rbase + hi * 8],
                    num_idxs=w * P,
                    num_idxs_reg=w * P,
                    elem_size=64,
                )
                R2v = R2S[:, lo * 64:hi * 64].rearrange(
                    "p (g c) -> p g c", c=64)[:, :, 0:19]
                eqc = eq2[:, lo * 19:hi * 19].rearrange(
                    "p (g c) -> p g c", c=19)
                nc.vector.tensor_tensor(out=eqc, in0=eqc, in1=R2v,
                                        op=mybir.AluOpType.mult)
                nc.vector.tensor_reduce(out=t1g[:, lo:hi], in_=eqc,
                                        axis=mybir.AxisListType.X,
                                        op=mybir.AluOpType.add)

            nc.sync.dma_start(out=out[:, :], in_=TG[:])

    nc.finalize()
    return nc


def _get_program(key):
    if key not in _programs:
        _programs[key] = _build_program(key)
    return _programs[key]


def _host_prep(inds_ref, inds_other):
    """Index-only host math: dedup scatter (last wins), split per core,
    sort both slot spaces by (two-pixel-row window, parity)."""
    ir = np.asarray(inds_ref).astype(np.int64)      # [B, 2, N]
    io = np.asarray(inds_other).astype(np.int64)
    valid = ((ir[:, 0] >= 0) & (ir[:, 0] < W) & (ir[:, 1] >= 0) & (ir[:, 1] < H)
             & (io[:, 0] >= 0) & (io[:, 0] < W) & (io[:, 1] >= 0)
             & (io[:, 1] < H))                       # [B, N]
    lin_ref = H * ir[:, 1] + ir[:, 0]                # [B, N]
    lin_other = H * io[:, 1] + io[:, 0]

    per_core = []
    count = 0
    for b in range(B):
        v = valid[b]
        lo = lin_other[b][v]
        lr = np.clip(lin_ref[b][v], 0, HW - 1)
        u, first_rev = np.unique(lo[::-1], return_index=True)
        d_arr = u.astype(np.int64)
        s_arr = lr[len(lo) - 1 - first_rev].astype(np.int64)
        count += len(u)
        for h in range(2):
            sel = (s_arr // NPIX_H) == h
            s_local = s_arr[sel] - h * NPIX_H
            d_sel = d_arr[sel]
            # group = (two-pixel-row window, pixel parity)
            gs = (s_local >> 16) * 2 + (s_local & 1)
            gd = (d_sel >> 16) * 2 + (d_sel & 1)
            s_ord = np.argsort(gs, kind='stable')
            d_ord = np.argsort(gd, kind='stable')
            per_core.append({
                "s": s_local[s_ord], "d": d_sel[d_ord],
                # for each s-sorted position, the d-sorted position of the
                # same correspondence (routing for the regather)
                "route": np.argsort(d_ord, kind='stable')[s_ord],
                "ngs": np.bincount(gs[s_ord], minlength=NG_S).astype(int),
                "ngd": np.bincount(gd[d_ord], minlength=NG_D).astype(int),
            })
    return per_core, count


def _plan(per_core):
    ngs = np.stack([pc["ngs"] for pc in per_core])
    ngd = np.stack([pc["ngd"] for pc in per_core])
    GS = np.maximum(1, -(-ngs.max(axis=0) // P))
    GD = np.maximum(1, -(-ngd.max(axis=0) // P))
    return (tuple(int(g) for g in GS), tuple(int(g) for g in GD))


def _wrap16(vals, ncols8):
    outp = np.zeros((16, ncols8), dtype=np.int16)
    j = np.arange(len(vals))
    outp[j % 16, j // 16] = vals.astype(np.int16)
    return outp


def _pack_core(pc, key):
    GS, GD = np.asarray(key[0]), np.asarray(key[1])
    Gs, Gd = int(GS.sum()), int(GD.sum())
    offs = np.concatenate([[0], np.cumsum(GS)]).astype(int)
    offd = np.concatenate([[0], np.cumsum(GD)]).astype(int)

    d_stream = np.zeros(Gd * P, dtype=np.int64)
    pm_d = np.zeros((P, Gd), dtype=np.float32)
    dpos2lin = np.zeros(len(pc["d"]), dtype=np.int64)
    pos = 0
    for g in range(NG_D):
        n = int(pc["ngd"][g])
        jj = np.arange(n)
        # window-local two-pixel row index
        d_stream[offd[g] * P + jj] = (pc["d"][pos:pos + n] >> 1) - \
            (g // 2) * W_ROWS
        gcol = offd[g] + jj // P
        pm_d[jj % P, gcol] = 1.0
        dpos2lin[pos:pos + n] = (jj % P) * Gd + gcol
        pos += n

    s_stream = np.zeros(Gs * P, dtype=np.int64)
    r_stream = np.zeros(Gs * P, dtype=np.int64)
    pm_s = np.zeros((P, Gs), dtype=np.float32)
    pos = 0
    for g in range(NG_S):
        n = int(pc["ngs"][g])
        jj = np.arange(n)
        s_stream[offs[g] * P + jj] = (pc["s"][pos:pos + n] >> 1) - \
            (g // 2) * W_ROWS
        r_stream[offs[g] * P + jj] = dpos2lin[pc["route"][pos:pos + n]]
        pm_s[jj % P, offs[g] + jj // P] = 1.0
        pos += n

    idx_d = np.tile(_wrap16(d_stream, Gd * 8), (8, 1))
    idx_sr = np.tile(np.concatenate([
        _wrap16(s_stream, Gs * 8),
        _wrap16(r_stream, Gs * 8),
    ], axis=1), (8, 1))
    pmio = np.concatenate([pm_d, pm_s], axis=1)
    return idx_d, idx_sr, pmio


def _make_in_maps(inputs_ref, inputs_other, per_core, key):
    ref_flat = inputs_ref.reshape(B, C, HW)
    other_flat = inputs_other.reshape(B, C, HW)
    other_cache = {}
    in_maps = []
    for ci, pc in enumerate(per_core):
        b, h = ci // 2, ci % 2
        ref64 = np.empty((NPIX_H, 64), dtype=np.float32)
        ref64[:, :C] = ref_flat[b, :, h * NPIX_H:(h + 1) * NPIX_H].T
        if b not in other_cache:
            o64 = np.empty((NPIX, 64), dtype=np.float32)
            o64[:, :C] = other_flat[b, :, :NPIX].T
            other_cache[b] = o64.reshape(NPIX // 2, E128)
        idx_d, idx_sr, pmio = _pack_core(pc, key)
        in_maps.append({
            "ref2": ref64.reshape(NPIX_H // 2, E128),
            "oth2": other_cache[b],
            "idx_d": idx_d,
            "idx_sr": idx_sr,
            "pmio": pmio,
        })
    return in_maps


def kernel(inputs_ref, inputs_other, inds_ref, inds_other, weights):
    from concourse.bass_utils import run_bass_kernel_spmd

    inputs_ref = np.asarray(inputs_ref, dtype=np.float32)
    inputs_other = np.asarray(inputs_other, dtype=np.float32)

    per_core, count = _host_prep(inds_ref, inds_other)
    key = _plan(per_core)
    nc = _get_program(key)

    in_maps = _make_in_maps(inputs_ref, inputs_other, per_core, key)
    res = run_bass_kernel_spmd(nc, in_maps, core_ids=list(range(NCORES)))
    total = 0.0
    Gs = sum(key[0])
    for r in res.results:
        o = np.asarray(r["out"], dtype=np.float64)
        total += o[:, :Gs].sum() - o[:, Gs:].sum()
    loss = -total / max(count, 1)
    return np.float32(loss)
